# revision 1
# baseline (speedup 1.0000x reference)
"""Trainium2 Bass kernel for ConvFourierKANLayer.

Computes y = conv2d(cos(x*k), w0) + conv2d(sin(x*k), w1) + bias for
k = 1..10 (G=10 Fourier orders), 3x3 kernel, pad 1, C=64 -> O=128.

Strategy (8 NeuronCores, data-parallel over batch B=16 -> 2 per core):
  - Host pre-transposes fouriercoeffs into 90 bf16 lhsT tiles
    [K=128, O=128] where K = (g_parity, c) packs two Fourier orders per
    matmul. Tiles are ordered j-major (j = order pair) so they stream
    from HBM in matmul-consumption order on the Act HWDGE queue while
    x loads ride the SP queue.
  - On-chip, x rows are expanded to cos/sin of k*x. The DVE has no fp
    mod, so the argument reduction uses the fp32 magic-number rounding
    trick (add/sub/mult only, fused into dual-ALU tensor_scalar ops):
        u  = x*(k/2pi) + 16        (positive; cos path uses +16.25)
        v  = (u + 2^23) - 2^23     (= round(u), one dual-ALU op)
        w  = u - v                 (in [-0.5, 0.5])
        sin(k*x) = Sin(w * 2pi)    (ScalarE spline, valid on [-pi, pi])
  - Implicit GEMM: per output strip, accumulate 90 bf16 matmuls
    (branch x g_pair x 3x3 taps) of [K=128]x[O=128] @ [K=128, N] into
    PSUM, N = 512*bankspan moving elements per matmul.
"""

import numpy as np

import concourse.bass as bass
import concourse.mybir as mybir
import concourse.tile as tile
from concourse import bacc
from concourse.bass_utils import run_bass_kernel_spmd

N_CORES = 8
B, C, H, W = 16, 64, 64, 64
O = 128
G = 10
BS = B // N_CORES  # batches per core
HT = 32  # output rows per chunk
NT = 2 * 5 * 9  # weight tiles: g_pair x branch x 3 x 3

PI = float(np.pi)
TWO_PI = float(2 * np.pi)
MAGIC = 8388608.0  # 2^23: fp32 round-to-nearest-integer magic constant

F32 = mybir.dt.float32

_CACHE = {}


def _build_module(reps=1, mmdt="bf16", ht=HT, bankspan=1, fused_round=True):
    MMDT = {
        "f32r": mybir.dt.float32r,
        "bf16": mybir.dt.bfloat16,
        "fp16": mybir.dt.float16,
    }[mmdt]
    rows = 8 * bankspan  # psum rows per matmul
    nb = ht // rows  # psum tiles per chunk
    nc = bacc.Bacc("TRN2", target_bir_lowering=False)
    x_d = nc.dram_tensor("x", [BS, C, H, W], F32, kind="ExternalInput")
    w_d = nc.dram_tensor("w", [128, NT, 128], MMDT, kind="ExternalInput")
    kv_d = nc.dram_tensor("kvec", [128, 5], F32, kind="ExternalInput")
    bias_d = nc.dram_tensor("biasv", [128, 1], F32, kind="ExternalInput")
    y_d = nc.dram_tensor("y", [BS, O, H, W], F32, kind="ExternalOutput")

    mult = mybir.AluOpType.mult
    add = mybir.AluOpType.add
    subtract = mybir.AluOpType.subtract
    sin_f = mybir.ActivationFunctionType.Sin

    with tile.TileContext(nc) as tc:
        with (
            tc.tile_pool(name="const", bufs=1) as cpool,
            tc.tile_pool(name="wpool", bufs=1) as wpool,
            tc.tile_pool(name="gen", bufs=2) as gen,
            tc.tile_pool(name="cspool", bufs=3) as cspool,
            tc.tile_pool(name="outp", bufs=3) as outp,
            tc.tile_pool(name="psum", bufs=2, space="PSUM") as psum,
        ):
            wt = wpool.tile([128, NT, 128], MMDT)
            for j in range(5):
                # j-major consumption order on the Act HWDGE queue
                nc.scalar.dma_start(
                    wt[:, 18 * j : 18 * (j + 1), :],
                    w_d[:, 18 * j : 18 * (j + 1), :],
                )
            kvt = cpool.tile([128, 5], F32)
            nc.sync.dma_start(kvt[:], kv_d[:])
            bt = cpool.tile([128, 1], F32)
            nc.sync.dma_start(bt[:], bias_d[:])

            for rep in range(reps):
              for b in range(BS):
                for h0 in range(0, H, ht):
                    gr0, gr1 = max(0, h0 - 1), min(H, h0 + ht + 1)
                    l0 = gr0 - (h0 - 1)  # local row index of first real row
                    nrows = gr1 - gr0
                    rs = slice(l0, l0 + nrows)

                    # split the x load at tile row m so the first compute
                    # half can start at half-DMA
                    m = l0 + 17
                    gm = gr0 + (m - l0)
                    xd = gen.tile([128, ht + 2, W], F32, tag="xdup")
                    nc.sync.dma_start(xd[0:64, l0:m, :], x_d[b, :, gr0:gm, :])
                    nc.sync.dma_start(xd[64:128, l0:m, :], x_d[b, :, gr0:gm, :])
                    nc.sync.dma_start(
                        xd[0:64, m : l0 + nrows, :], x_d[b, :, gm:gr1, :]
                    )
                    nc.sync.dma_start(
                        xd[64:128, m : l0 + nrows, :], x_d[b, :, gm:gr1, :]
                    )

                    pss = [
                        psum.tile([128, rows, 64], F32, tag=f"ps{bk}",
                                  name=f"ps{bk}_{rep}_{b}_{h0}")
                        for bk in range(nb)
                    ]

                    for j in range(5):
                        # u = x*(k/2pi) + 16 ; v = round(u) ; w = u - v
                        # (cos path phase-shifts u by a quarter turn)
                        us = gen.tile([128, ht + 2, W], F32, tag="us")
                        uc = gen.tile([128, ht + 2, W], F32, tag="uc")
                        st = cspool.tile([128, ht + 2, W + 2], MMDT, tag="ss")
                        ct = cspool.tile([128, ht + 2, W + 2], MMDT, tag="cs")
                        vs = gen.tile([128, ht + 2, W], F32, tag="vt", bufs=2)
                        vc = gen.tile([128, ht + 2, W], F32, tag="vt", bufs=2)
                        ws = gen.tile([128, ht + 2, W], F32, tag="wt")
                        wc = gen.tile([128, ht + 2, W], F32, tag="wt")

                        # zero conv borders of the matmul input tiles
                        if mmdt == "f32r":
                            u32 = mybir.dt.uint32
                            zb = lambda ap: ap.bitcast(u32)
                        else:
                            zb = lambda ap: ap
                        for z in (st, ct):
                            nc.gpsimd.memset(zb(z[:, :, 0:1]), 0)
                            nc.gpsimd.memset(zb(z[:, :, W + 1 : W + 2]), 0)
                            if l0 == 1:
                                nc.gpsimd.memset(zb(z[:, 0:1, :]), 0)
                            if gr1 == H:
                                nc.gpsimd.memset(
                                    zb(z[:, ht + 1 : ht + 2, :]), 0
                                )

                        # row-split the very first pipeline so compute
                        # overlaps the first x DMA
                        if b == 0 and h0 == 0 and j == 0:
                            ranges = [(l0, m), (m, l0 + nrows)]
                        else:
                            ranges = [(l0, l0 + nrows)]
                        for r0, r1 in ranges:
                            r = slice(r0, r1)
                            for u_t, v_t, w_t, z, ph in (
                                (us, vs, ws, st, 16.0),
                                (uc, vc, wc, ct, 16.25),
                            ):
                                nc.vector.tensor_scalar(
                                    u_t[:, r, :], xd[:, r, :],
                                    kvt[:, j : j + 1], ph, mult, add,
                                )
                                # v = (u + 2^23) - 2^23 in one dual-ALU op
                                nc.vector.tensor_scalar(
                                    v_t[:, r, :], u_t[:, r, :],
                                    MAGIC, MAGIC, add, subtract,
                                )
                                nc.vector.tensor_sub(
                                    w_t[:, r, :], u_t[:, r, :], v_t[:, r, :]
                                )
                                nc.scalar.activation(
                                    z[:, r, 1 : W + 1], w_t[:, r, :], sin_f,
                                    scale=TWO_PI,
                                )

                        for br in range(2):
                            src = st if br == 0 else ct
                            for dh in range(3):
                                for dw in range(3):
                                    t_idx = ((j * 2 + br) * 3 + dh) * 3 + dw
                                    for bk in range(nb):
                                        nc.tensor.matmul(
                                            pss[bk][:],
                                            wt[:, t_idx, :],
                                            src[
                                                :,
                                                rows * bk + dh
                                                : rows * bk + dh + rows,
                                                dw : dw + 64,
                                            ],
                                            start=(j == 0 and br == 0
                                                   and dh == 0 and dw == 0),
                                            stop=(j == 4 and br == 1
                                                  and dh == 2 and dw == 2),
                                        )

                    for bk in range(nb):
                        ob = outp.tile([128, rows, 64], F32, tag="ob")
                        nc.vector.tensor_scalar_add(ob[:], pss[bk][:], bt[:, 0:1])
                        # alternate queues so stores drain in parallel
                        eng = nc.scalar if bk % 2 == 0 else nc.sync
                        eng.dma_start(
                            y_d[b, :, h0 + rows * bk : h0 + rows * (bk + 1), :],
                            ob[:],
                        )
    nc.finalize()
    return nc


def _get_module(reps=1, mmdt="bf16", ht=HT, bankspan=1, fused_round=True):
    key = ("nc", reps, mmdt, ht, bankspan, fused_round)
    if key not in _CACHE:
        _CACHE[key] = _build_module(reps, mmdt, ht, bankspan, fused_round)
    return _CACHE[key]


def _np_mmdt(mmdt):
    import ml_dtypes
    return {"f32r": np.float32, "bf16": ml_dtypes.bfloat16,
            "fp16": np.float16}[mmdt]


def _host_weights(fc, mmdt="bf16"):
    # fc: (2, O, C, kH, kW, G) -> w[p=(gp*64+c), t=(j,br,kh,kw), o]
    # br=0 is the SIN branch (fouriercoeffs[1]): its on-chip tile is
    # ready first, so it leads each j-group's matmuls.
    fc = fc[::-1]
    W6 = np.transpose(fc, (0, 5, 3, 4, 2, 1))  # (br, g, kh, kw, c, o)
    W6 = W6.reshape(2, 5, 2, 3, 3, 64, 128)  # (br, j, gp, kh, kw, c, o)
    Wt = np.transpose(W6, (1, 0, 3, 4, 2, 5, 6))  # (j, br, kh, kw, gp, c, o)
    Wt = Wt.reshape(NT, 128, 128)
    return np.ascontiguousarray(
        np.transpose(Wt, (1, 0, 2)).astype(_np_mmdt(mmdt))
    )


def _host_kvec():
    kvec = np.zeros((128, 5), np.float32)
    for j in range(5):
        kvec[0:64, j] = (2 * j + 1) / TWO_PI
        kvec[64:128, j] = (2 * j + 2) / TWO_PI
    return kvec


def kernel(x, fouriercoeffs, bias):
    x = np.ascontiguousarray(np.asarray(x, dtype=np.float32))
    fc = np.asarray(fouriercoeffs, dtype=np.float32)
    w_host = _host_weights(fc)
    kvec = _host_kvec()
    biasv = np.ascontiguousarray(
        np.asarray(bias, dtype=np.float32).reshape(128, 1)
    )

    nc = _get_module()
    in_maps = [
        {"x": x[i * BS : (i + 1) * BS], "w": w_host, "kvec": kvec, "biasv": biasv}
        for i in range(N_CORES)
    ]
    res = run_bass_kernel_spmd(nc, in_maps, list(range(N_CORES))).results
    return np.concatenate([res[i]["y"] for i in range(N_CORES)], axis=0)

